# revision 1
# baseline (speedup 1.0000x reference)
"""AttentionPairBias TRN2 kernel — 8-core SPMD, query-row sharding, bf16.

Per core (q-block = 128 rows):
  - host folds LN(s) scale/bias and 1/sqrt(HD) into projection weights; folds
    the z-LN mean term into Wz (W'' = diag(w)Wz - ones*colsum/DZ; the b@Wz row
    is softmax-invariant and dropped).  z shard is pre-transposed AND pre-cast
    to bf16: zT16 [c=128, k=1024, q=128].
  - device streams zT16 in 32 chunks of 32 k-planes.  Per k-plane one bf16
    matmul (stationary z-plane, moving [W''|1/DZ] 17 cols) gives zb_raw and
    the mean; a second 1-col matmul on the squared plane gives E[z^2].
    rstd = exp(-0.5*ln(var+eps)) on ACT (ln+exp share one activation table
    set -> no table churn); zb written bf16 in [q, h, k] layout.
  - attention is fused per 128-k block (flash style, no max subtraction:
    args are small): scores psum = QK^T (4 heads packed per psum bank via
    tile_position row groups) + zb via identity matmul; single exp per
    4-head group on ACT; probs transposed on PE; o and the softmax rowsums
    (probs^T @ ones) accumulate in dedicated psum banks across all 8 blocks
    with exactly ONE start=True per bank (a start clears has_written for
    the whole bank); normalization deferred to the output (o * 1/rowsum).
  - program order: two blocks of z chunks first (so the z DMA stream starts
    at t=0 with runway), then the s path (LN + QKVG projections), then
    fused attention running two blocks behind the z stream;
    out rows = (o*g*rec) @ Wo; host concatenates the 8 row-shards.
"""
import sys, os
sys.path.insert(0, "/opt/trn_rl_repo")
import numpy as np

import concourse.bass as bass
import concourse.bacc as bacc
import concourse.mybir as mybir
import concourse.tile as tile
from concourse.bass_utils import run_bass_kernel_spmd

F32 = mybir.dt.float32
BF16 = mybir.dt.bfloat16
AF = mybir.ActivationFunctionType
OP = mybir.AluOpType
BF = mybir.dt.np(mybir.dt.bfloat16)

B, N, H, HD, D, DZ = 1, 1024, 16, 32, 512, 128
NC = 8
NQ = N // NC          # 128 q rows per core
KC = 32               # k-planes per DMA chunk
BLK = 128             # k per fused attention block
CPB = BLK // KC       # chunks per block
NBLK = N // BLK
EPS = 1e-5

_CACHED = None


def _build():
    nc = bacc.Bacc(None, target_bir_lowering=False)

    st_d = nc.dram_tensor("s_t", [128, 8, D], F32, kind="ExternalInput")
    sq_d = nc.dram_tensor("s_q", [NQ, D], F32, kind="ExternalInput")
    zT_d = nc.dram_tensor("zT16", [DZ, N, NQ], BF16, kind="ExternalInput")
    wq_d = nc.dram_tensor("Wq", [128, 4, D], BF16, kind="ExternalInput")
    wk_d = nc.dram_tensor("Wk", [128, 4, D], BF16, kind="ExternalInput")
    wv_d = nc.dram_tensor("Wv", [128, 4, D], BF16, kind="ExternalInput")
    wg_d = nc.dram_tensor("Wg", [128, 4, D], BF16, kind="ExternalInput")
    wo_d = nc.dram_tensor("Wo", [128, 4, D], BF16, kind="ExternalInput")
    bq_d = nc.dram_tensor("bq_t", [128, 4], F32, kind="ExternalInput")
    bk_d = nc.dram_tensor("bk_t", [128, 4], F32, kind="ExternalInput")
    bv_d = nc.dram_tensor("bv", [D], F32, kind="ExternalInput")
    bg_d = nc.dram_tensor("bg", [D], F32, kind="ExternalInput")
    wext_d = nc.dram_tensor("Wext", [DZ, 17], BF16, kind="ExternalInput")
    ones_d = nc.dram_tensor("onescol", [DZ, 1], BF16, kind="ExternalInput")
    id_d = nc.dram_tensor("ident", [128, 128], BF16, kind="ExternalInput")
    out_d = nc.dram_tensor("out", [NQ, D], F32, kind="ExternalOutput")
    DEBUG = bool(os.environ.get("K_DEBUG"))
    if DEBUG:
        zb_d = nc.dram_tensor("zb_dbg", [NQ, H, N], BF16, kind="ExternalOutput")
        kt_d = nc.dram_tensor("kt_dbg", [128, N], BF16, kind="ExternalOutput")
        v_d = nc.dram_tensor("v_dbg", [128, D], BF16, kind="ExternalOutput")
        q_d = nc.dram_tensor("q_dbg", [128, NQ], BF16, kind="ExternalOutput")
        g_d = nc.dram_tensor("g_dbg", [128, D], F32, kind="ExternalOutput")
        rs_d = nc.dram_tensor("rs_dbg", [128, H], F32, kind="ExternalOutput")
        o_d = nc.dram_tensor("o_dbg", [128, D], F32, kind="ExternalOutput")

    with tile.TileContext(nc) as tc:
        with tc.tile_pool(name="const", bufs=1) as cpool, \
             tc.tile_pool(name="persist", bufs=1) as pp, \
             tc.tile_pool(name="sA", bufs=2) as ap_, \
             tc.tile_pool(name="zB", bufs=6) as zp, \
             tc.tile_pool(name="z2B", bufs=4) as z2p, \
             tc.tile_pool(name="aB", bufs=6) as abuf, \
             tc.tile_pool(name="attC", bufs=3) as att, \
             tc.tile_pool(name="psO", bufs=1, space="PSUM") as psO, \
             tc.tile_pool(name="psR", bufs=1, space="PSUM") as psR, \
             tc.tile_pool(name="psG", bufs=3, space="PSUM") as psG, \
             tc.tile_pool(name="psS", bufs=2, space="PSUM") as psS, \
             tc.tile_pool(name="psE", bufs=1, space="PSUM") as psE:

            # ---------------- constants / weights (gpsimd DMA queue) --------
            ident = cpool.tile([128, 128], BF16)
            nc.gpsimd.dma_start(out=ident, in_=id_d[:, :])
            wext = cpool.tile([DZ, 17], BF16)
            nc.gpsimd.dma_start(out=wext, in_=wext_d[:, :])
            onescol = cpool.tile([DZ, 1], BF16)
            nc.gpsimd.dma_start(out=onescol, in_=ones_d[:, :])
            eps_t = cpool.tile([128, 1], F32)
            nc.vector.memset(eps_t, EPS)
            ones1 = cpool.tile([128, 1], BF16)
            nc.vector.memset(ones1, 1.0)
            bq_t = cpool.tile([128, 4], F32)
            nc.gpsimd.dma_start(out=bq_t, in_=bq_d[:, :])
            bk_t = cpool.tile([128, 4], F32)
            nc.gpsimd.dma_start(out=bk_t, in_=bk_d[:, :])
            bg_rep = cpool.tile([128, D], F32)
            bg_ap = bg_d[:]
            nc.gpsimd.dma_start(
                out=bg_rep,
                in_=bass.AP(tensor=bg_ap.tensor, offset=bg_ap.offset,
                            ap=[[0, 128], [1, D]]))
            bv_rep = cpool.tile([128, D], F32)
            bv_ap = bv_d[:]
            nc.gpsimd.dma_start(
                out=bv_rep,
                in_=bass.AP(tensor=bv_ap.tensor, offset=bv_ap.offset,
                            ap=[[0, 128], [1, D]]))
            wk = cpool.tile([128, 4, D], BF16)
            nc.gpsimd.dma_start(out=wk, in_=wk_d[:, :, :])
            wv = cpool.tile([128, 4, D], BF16)
            nc.gpsimd.dma_start(out=wv, in_=wv_d[:, :, :])
            wq = cpool.tile([128, 4, D], BF16)
            nc.gpsimd.dma_start(out=wq, in_=wq_d[:, :, :])
            wg = cpool.tile([128, 4, D], BF16)
            nc.gpsimd.dma_start(out=wg, in_=wg_d[:, :, :])
            wo = cpool.tile([128, 4, D], BF16)
            nc.gpsimd.dma_start(out=wo, in_=wo_d[:, :, :])
            sts = cpool.tile([128, 8, D], F32)
            nc.gpsimd.dma_start(out=sts, in_=st_d[:, :, :])
            stq = cpool.tile([128, D], F32)
            nc.gpsimd.dma_start(out=stq, in_=sq_d[:, :])

            # ---------------- persistent activation storage -----------------
            slnT = pp.tile([128, 4, N], BF16)         # (d%128, dtile, tok)
            sqT = pp.tile([128, 4, NQ], BF16)         # (d%128, dtile, q)
            KT = [pp.tile([128, N], BF16, name=f"KT{b}") for b in range(4)]
            Vt = [pp.tile([128, D], BF16, name=f"V{t}") for t in range(8)]
            QT = [pp.tile([128, NQ], BF16, name=f"QT{b}") for b in range(4)]
            G_sb = pp.tile([128, D], F32, name="G_sb")
            zb = pp.tile([128, H, N], BF16, name="zb")    # (q, h, k)
            o_ps = psO.tile([128, D], F32, name="o_ps")
            rs_ps = psR.tile([128, H], F32, name="rs_ps")

            # ---------------- z-chunk pipeline -------------------------------
            def z_chunk(ci):
                z16 = zp.tile([128, KC, 128], BF16, tag="z16")
                nc.sync.dma_start(out=z16,
                                  in_=zT_d[:, ci * KC:(ci + 1) * KC, :])
                z2 = z2p.tile([128, KC, 128], BF16, tag="z2")
                f_in = z16.rearrange("c k q -> c (k q)")
                f_out = z2.rearrange("c k q -> c (k q)")
                hf = KC * 128 // 2
                nc.vector.tensor_mul(f_out[:, 0:hf], f_in[:, 0:hf],
                                     f_in[:, 0:hf])
                nc.gpsimd.tensor_mul(f_out[:, hf:], f_in[:, hf:],
                                     f_in[:, hf:])
                for g in range(2):
                    kb = ci * KC + g * 16
                    ps = psG.tile([128, 512], F32, tag="zg")
                    for j in range(16):
                        kk = g * 16 + j
                        nc.tensor.matmul(ps[:, j * 17:(j + 1) * 17],
                                         z16[:, kk, :], wext,
                                         start=True, stop=True)
                        nc.tensor.matmul(ps[:, 272 + j:273 + j],
                                         z2[:, kk, :], onescol,
                                         start=True, stop=True)
                    raw3 = ps[:, 0:272].rearrange("p (k c) -> p k c", c=17)
                    mus = raw3[:, :, 16]
                    mu2 = abuf.tile([128, 16], F32, tag="mu2")
                    nc.scalar.square(mu2, mus)
                    var = abuf.tile([128, 16], F32, tag="var")
                    nc.vector.tensor_sub(var, ps[:, 272:288], mu2)
                    lnv = abuf.tile([128, 16], F32, tag="lnv")
                    nc.scalar.activation(out=lnv, in_=var, func=AF.Ln,
                                         bias=eps_t)
                    alpha = abuf.tile([128, 16], F32, tag="alpha")
                    nc.scalar.activation(out=alpha, in_=lnv, func=AF.Exp,
                                         scale=-0.5)
                    alpha_b = bass.AP(
                        tensor=alpha.tensor, offset=alpha.offset,
                        ap=[list(alpha.ap[0]), [0, 16], list(alpha.ap[1])])
                    nc.vector.tensor_mul(
                        zb[:, :, kb:kb + 16],
                        raw3[:, :, 0:16].rearrange("p k h -> p h k"),
                        alpha_b)

            # ---------------- fused attention block --------------------------
            def att_block(blk):
                ks = slice(blk * BLK, (blk + 1) * BLK)
                for b in range(4):
                    ps_sc = psS.tile([128, 512], F32, tag="sc")
                    for r in range(4):
                        h = b * 4 + r
                        cs = slice(r * 128, (r + 1) * 128)
                        rs_ = slice(r * 32, (r + 1) * 32)
                        nc.tensor.matmul(ps_sc[:, cs], QT[b][rs_, :],
                                         KT[b][rs_, ks],
                                         start=True, stop=False,
                                         tile_position=(r * 32, 0))
                        nc.tensor.matmul(ps_sc[:, cs], ident, zb[:, h, ks],
                                         start=False, stop=True)
                    e16 = att.tile([128, 512], BF16, tag="e16")
                    nc.scalar.activation(out=e16, in_=ps_sc, func=AF.Exp)
                    ps_eT = psE.tile([128, 512], BF16, tag="eT")
                    for r in range(4):
                        cs = slice(r * 128, (r + 1) * 128)
                        nc.tensor.transpose(ps_eT[:, cs], e16[:, cs], ident)
                    eTs = att.tile([128, 512], BF16, tag="eTs")
                    if b % 2 == 0:
                        nc.scalar.copy(eTs, ps_eT)
                    else:
                        nc.vector.tensor_copy(eTs, ps_eT)
                    for r in range(4):
                        h = b * 4 + r
                        cs = slice(r * 128, (r + 1) * 128)
                        os_ = slice(h * 32, (h + 1) * 32)
                        first = (blk == 0 and h == 0)
                        last = (blk == NBLK - 1 and h == H - 1)
                        # exactly ONE start=True per accumulator bank: a
                        # start clears has_written for the WHOLE bank, so a
                        # per-head start would wipe the other heads' bits and
                        # later blocks would overwrite instead of accumulate.
                        nc.tensor.matmul(o_ps[:, os_], eTs[:, cs],
                                         Vt[blk][:, os_],
                                         start=first, stop=last,
                                         skip_group_check=True)
                        nc.tensor.matmul(rs_ps[:, h:h + 1], eTs[:, cs],
                                         ones1,
                                         start=first, stop=last,
                                         skip_group_check=True)

            # ---------------- s path (LN + projections) ----------------------
            def layernorm_tile(src_ap, tag):
                stats = ap_.tile([128, 6], F32, tag="stats", name=f"st_{tag}")
                nc.vector.bn_stats(out=stats, in_=src_ap)
                mv = ap_.tile([128, 2], F32, tag="mv", name=f"mv{tag}")
                nc.vector.bn_aggr(out=mv, in_=stats)
                lnv = ap_.tile([128, 1], F32, tag="lnv", name=f"lnv{tag}")
                nc.scalar.activation(out=lnv, in_=mv[:, 1:2], func=AF.Ln,
                                     bias=eps_t)
                rst = ap_.tile([128, 1], F32, tag="rst", name=f"rst{tag}")
                nc.scalar.activation(out=rst, in_=lnv, func=AF.Exp,
                                     scale=-0.5)
                sln = ap_.tile([128, D], BF16, tag="sln", name=f"sln{tag}")
                nc.vector.scalar_tensor_tensor(
                    out=sln, in0=src_ap, scalar=mv[:, 0:1],
                    in1=rst.to_broadcast((128, D)),
                    op0=OP.subtract, op1=OP.mult)
                return sln

            def phase_A():
                # full-s LN + transpose into slnT
                for t in range(8):
                    sln = layernorm_tile(sts[:, t, :], f"s{t}")
                    ps = psE.tile([128, D], BF16, tag="eT")
                    for j in range(4):
                        nc.tensor.transpose(ps[:, j * 128:(j + 1) * 128],
                                            sln[:, j * 128:(j + 1) * 128],
                                            ident)
                    nc.scalar.copy(
                        slnT[:, :, t * 128:(t + 1) * 128],
                        ps.rearrange("p (j q) -> p j q", j=4))
                # q-block LN + transpose into sqT
                slnq = layernorm_tile(stq[:, :], "q")
                psq = psE.tile([128, D], BF16, tag="eT")
                for j in range(4):
                    nc.tensor.transpose(psq[:, j * 128:(j + 1) * 128],
                                        slnq[:, j * 128:(j + 1) * 128], ident)
                nc.scalar.copy(sqT[:, :, :],
                               psq.rearrange("p (j q) -> p j q", j=4))

                # KT[b] = (sln @ Wk + bk)^T  -> [hd(128b), tok]
                for b in range(4):
                    bs = slice(b * 128, (b + 1) * 128)
                    for half in range(2):
                        hs = slice(half * 512, (half + 1) * 512)
                        ps = psS.tile([128, 512], F32, tag="sc")
                        for dt_ in range(4):
                            nc.tensor.matmul(ps, wk[:, dt_, bs],
                                             slnT[:, dt_, hs],
                                             start=(dt_ == 0), stop=(dt_ == 3))
                        nc.scalar.activation(out=KT[b][:, hs], in_=ps,
                                             func=AF.Identity,
                                             bias=bk_t[:, b:b + 1], scale=1.0)
                # V[t] = sln @ Wv + bv  (natural [tok, hd])
                for t in range(8):
                    ts = slice(t * 128, (t + 1) * 128)
                    ps = psS.tile([128, 512], F32, tag="sc")
                    for dt_ in range(4):
                        nc.tensor.matmul(ps, slnT[:, dt_, ts], wv[:, dt_, :],
                                         start=(dt_ == 0), stop=(dt_ == 3))
                    nc.vector.tensor_add(Vt[t], ps, bv_rep)
                # QT[b] from the q-block
                for b in range(4):
                    bs = slice(b * 128, (b + 1) * 128)
                    psqt = psS.tile([128, NQ], F32, tag="sc")
                    for dt_ in range(4):
                        nc.tensor.matmul(psqt, wq[:, dt_, bs], sqT[:, dt_, :],
                                         start=(dt_ == 0), stop=(dt_ == 3))
                    nc.scalar.activation(out=QT[b], in_=psqt, func=AF.Identity,
                                         bias=bq_t[:, b:b + 1], scale=1.0)
                # G = sigmoid(s@Wg + bg) = 1/(1+exp(-x)) (stays in exp set)
                psg = psS.tile([128, D], F32, tag="sc")
                for dt_ in range(4):
                    nc.tensor.matmul(psg, sqT[:, dt_, :], wg[:, dt_, :],
                                     start=(dt_ == 0), stop=(dt_ == 3))
                gx = ap_.tile([128, D], F32, tag="sln", name="gx")
                nc.vector.tensor_add(gx, psg, bg_rep)
                ge = ap_.tile([128, D], F32, tag="sln", name="ge")
                nc.scalar.activation(out=ge, in_=gx, func=AF.Exp, scale=-1.0)
                gd = ap_.tile([128, D], F32, tag="sln", name="gd")
                nc.vector.tensor_scalar_add(gd, ge, 1.0)
                nc.vector.reciprocal(G_sb, gd)

            # ---------------- program order ----------------------------------
            # two blocks of z-stream runway before the s path, and attention
            # two blocks behind the stream, so the PE never starves the DMA
            for blk in range(2):
                for ch in range(CPB):
                    z_chunk(blk * CPB + ch)
            phase_A()
            for blk in range(2, NBLK):
                for ch in range(CPB):
                    z_chunk(blk * CPB + ch)
                att_block(blk - 2)
            att_block(NBLK - 2)
            att_block(NBLK - 1)

            # ---------------- epilogue ---------------------------------------
            with tc.tile_pool(name="fin", bufs=1) as fin:
                if DEBUG:
                    nc.sync.dma_start(out=zb_d[:, :, :], in_=zb)
                    nc.sync.dma_start(out=kt_d[:, :], in_=KT[0])
                    nc.sync.dma_start(out=v_d[:, :], in_=Vt[0])
                    nc.sync.dma_start(out=q_d[:, :], in_=QT[0])
                    nc.sync.dma_start(out=g_d[:, :], in_=G_sb)
                    rs_c = fin.tile([128, H], F32, name="rs_c")
                    nc.scalar.copy(rs_c, rs_ps)
                    nc.sync.dma_start(out=rs_d[:, :], in_=rs_c)
                    o_c = fin.tile([128, D], F32, name="o_c")
                    nc.scalar.copy(o_c, o_ps)
                    nc.sync.dma_start(out=o_d[:, :], in_=o_c)
                rec = fin.tile([128, H], F32)
                nc.vector.reciprocal(rec, rs_ps)
                t1 = fin.tile([128, D], F32)
                nc.vector.tensor_mul(t1, o_ps, G_sb)
                og = fin.tile([128, D], BF16)
                rec_b = bass.AP(
                    tensor=rec.tensor, offset=rec.offset,
                    ap=[list(rec.ap[0]), list(rec.ap[1]), [0, HD]])
                nc.vector.tensor_mul(og, t1, rec_b)
                ps_tr = psE.tile([128, D], BF16, tag="eT")
                for gidx in range(4):
                    cs = slice(gidx * 128, (gidx + 1) * 128)
                    nc.tensor.transpose(ps_tr[:, cs], og[:, cs], ident)
                ogT = fin.tile([128, D], BF16)
                nc.scalar.copy(ogT, ps_tr)
                ps_out = psS.tile([128, D], F32, tag="sc")
                for gidx in range(4):
                    cs = slice(gidx * 128, (gidx + 1) * 128)
                    nc.tensor.matmul(ps_out, ogT[:, cs], wo[:, gidx, :],
                                     start=(gidx == 0), stop=(gidx == 3))
                out_sb = fin.tile([128, D], F32)
                nc.scalar.copy(out_sb, ps_out)
                nc.sync.dma_start(out=out_d[:, :], in_=out_sb)

    nc.compile()
    return nc


def _get_nc():
    global _CACHED
    if _CACHED is None:
        _CACHED = _build()
    return _CACHED


def _prepare_inputs(s, z, norm_s_w, norm_s_b, Wq, bq, Wk, Wv, Wg,
                    z_norm_w, z_norm_b, Wz, Wo):
    s2 = np.asarray(s, np.float32).reshape(N, D)
    z3 = np.asarray(z, np.float32).reshape(N, N, DZ)
    w_s = np.asarray(norm_s_w, np.float32)
    b_s = np.asarray(norm_s_b, np.float32)
    scale = np.float32(HD ** -0.5)
    Wq_f = (w_s[:, None] * np.asarray(Wq, np.float32)) * scale
    bq_f = (np.asarray(bq, np.float32) + b_s @ np.asarray(Wq, np.float32)) * scale
    Wk_f = w_s[:, None] * np.asarray(Wk, np.float32)
    bk_f = b_s @ np.asarray(Wk, np.float32)
    Wv_f = w_s[:, None] * np.asarray(Wv, np.float32)
    bv_f = b_s @ np.asarray(Wv, np.float32)
    Wg_f = w_s[:, None] * np.asarray(Wg, np.float32)
    bg_f = b_s @ np.asarray(Wg, np.float32)
    Wp = np.asarray(z_norm_w, np.float32)[:, None] * np.asarray(Wz, np.float32)
    S = Wp.sum(0)
    Wpp = Wp - np.ones((DZ, 1), np.float32) @ (S[None, :] / DZ)
    wext = np.ascontiguousarray(np.concatenate(
        [Wpp, np.full((DZ, 1), 1.0 / DZ, np.float32)], 1)).astype(BF)
    onescol = np.full((DZ, 1), 1.0 / DZ, BF)
    ident = np.eye(128, dtype=np.float32).astype(BF)

    def tile_w(W):
        return np.ascontiguousarray(
            W.reshape(4, 128, D).transpose(1, 0, 2)).astype(BF)

    s_t = np.ascontiguousarray(s2.reshape(8, 128, D).transpose(1, 0, 2))
    shared = {
        "s_t": s_t,
        "Wq": tile_w(Wq_f), "Wk": tile_w(Wk_f), "Wv": tile_w(Wv_f),
        "Wg": tile_w(Wg_f), "Wo": tile_w(np.asarray(Wo, np.float32)),
        "bq_t": np.ascontiguousarray(bq_f.reshape(4, 128).T),
        "bk_t": np.ascontiguousarray(bk_f.reshape(4, 128).T),
        "bv": np.ascontiguousarray(bv_f), "bg": np.ascontiguousarray(bg_f),
        "Wext": wext, "onescol": onescol, "ident": ident,
    }
    in_maps = []
    for c in range(NC):
        qs = slice(c * NQ, (c + 1) * NQ)
        zTc = np.ascontiguousarray(z3[qs].transpose(2, 1, 0).astype(BF))
        m = dict(shared)
        m["s_q"] = np.ascontiguousarray(s2[qs])
        m["zT16"] = zTc
        in_maps.append(m)
    return in_maps


def _run(in_maps, trace=False):
    nc = _get_nc()
    return run_bass_kernel_spmd(nc, in_maps, core_ids=list(range(NC)),
                                trace=trace)


def kernel(**inputs):
    in_maps = _prepare_inputs(**inputs)
    res = _run(in_maps, trace=False)
    out = np.concatenate([res.results[c]["out"] for c in range(NC)], 0)
    return out.reshape(B, N, D).astype(np.float32)

